# revision 7
# baseline (speedup 1.0000x reference)
"""Trainium2 Bass kernel for GQA attention block (nn_Attention_81372450390110).

Module: y = AttnOut(x) with q/k RMSNorm + interleaved RoPE + causal GQA
(NH=16 q heads, KVH=4 kv heads, HD=128, D=2048, B=2, S=2048).

Sharding: 8 cores = 2 batches x 4 KV groups. Core c handles batch c//4 and
KV group c%4 (4 q heads + 1 kv head). Each core computes a full [S, D]
partial of the output projection (row-parallel over heads); the host sums
the 4 group-partials per batch.

Layout strategy (all feature-major, "transposed"):
  - host passes xT = x[b].T so the D contraction dim lands on partitions
  - qT/kT computed as [HD, S] directly (lhsT = weight chunk)
  - scores computed transposed: sT[k, q] = kT_blk.T @ qT_blk
  - softmax without max-subtraction (rmsnorm bounds |scores| <= sqrt(HD))
  - P@V computed transposed with V as the stationary operand:
      attT[hd, q] += v_blk.T.T @ pT_blk   (N=512 streams, no PE transposes)
    The softmax denominator l[q] accumulates in a parallel PSUM bank via an
    all-ones [128,128] stationary (every output partition = l), so the
    normalize is a single DVE mul with reciprocal_approx_fast - no
    broadcast matmul, no per-128-block transpose/copy chain.
  - rmsnorm row-sums use the same all-ones-stationary trick (ssq broadcast
    to all partitions in one matmul); rsqrt = Exp(-0.5*Ln(x)) so the whole
    kernel lives in one ACT table set (natural_log_exp_and_others), which
    lets projection rounds and attention rounds interleave without table
    reloads.
  - rounds pipeline: [oproj(nb-1)] [proj block nb + norm/rope] [attn qt=nb]
    so the PE never sees a phase barrier; x-block DMA for round nb+1
    overlaps attention nb.
  - RoPE+norm-weight folded into host-precomputed coefficient tiles, with
    an even/odd deinterleaving permutation baked into wq/wk columns
"""

import os
import sys

sys.path.insert(0, "/opt/trn_rl_repo")

import numpy as np
import ml_dtypes

BF16 = ml_dtypes.bfloat16

B = 2
S = 2048
D = 2048
NH = 16
KVH = 4
HD = 128
THETA = 10000.0
EPS = 1e-6
NHL = NH // KVH  # q heads per core (4)
SCALE = 1.0 / float(np.sqrt(HD))

_CACHED = {}


def build_nc(s=S, d=D, nhl=NHL, hd=HD):
    import concourse.mybir as mybir
    import concourse.tile as tile
    from concourse import bacc

    f32 = mybir.dt.float32
    bf16 = mybir.dt.bfloat16
    AF = mybir.ActivationFunctionType

    kc_n = d // 128          # contraction chunks for projections
    nb_n = s // 512          # 512-token blocks / rounds
    kb_n = s // 128          # k blocks (128 wide)

    nc = bacc.Bacc("TRN2", target_bir_lowering=False, debug=False)

    xT_d = nc.dram_tensor("xT", (d, s), bf16, kind="ExternalInput")
    wq_d = nc.dram_tensor("wq", (d, nhl * hd), bf16, kind="ExternalInput")
    wk_d = nc.dram_tensor("wk", (d, hd), bf16, kind="ExternalInput")
    wv_d = nc.dram_tensor("wv", (d, hd), bf16, kind="ExternalInput")
    wo_d = nc.dram_tensor("wo", (nhl * hd, d), bf16, kind="ExternalInput")
    m1q_d = nc.dram_tensor("m1q", (hd, s), bf16, kind="ExternalInput")
    m2q_d = nc.dram_tensor("m2q", (hd, s), bf16, kind="ExternalInput")
    m1k_d = nc.dram_tensor("m1k", (hd, s), bf16, kind="ExternalInput")
    m2k_d = nc.dram_tensor("m2k", (hd, s), bf16, kind="ExternalInput")
    tri_d = nc.dram_tensor("tri", (128, 128), bf16, kind="ExternalInput")
    y_d = nc.dram_tensor("y", (s, d), f32, kind="ExternalOutput")

    with tile.TileContext(nc) as tc, nc.allow_low_precision(
        reason="bf16 compute by design; fp32 accumulation in PSUM"
    ):
        with (
            tc.tile_pool(name="const", bufs=1) as const,
            tc.tile_pool(name="persist", bufs=1) as persist,
            tc.tile_pool(name="xtp", bufs=2) as xtp,
            tc.tile_pool(name="workA", bufs=3) as wa,
            tc.tile_pool(name="workB", bufs=3) as wb,
            tc.tile_pool(name="ps", bufs=1, space="PSUM") as ps,
        ):
            # Pin the ACT table set to natural_log_exp_and_others (id 6:
            # square+ln+exp) up front; the insert_act_table_loads fixpoint
            # then sees every activation covered and emits no further
            # (thrashing) reloads.
            nc.scalar.add_instruction(
                mybir.InstLoadActFuncSet(
                    name=nc.get_next_instruction_name(),
                    act_func_set_id=6,
                    ins=[], outs=[],
                )
            )

            # ---- constants -----------------------------------------------
            ones128 = const.tile([128, 128], bf16, tag="ones128")
            nc.vector.memset(ones128[:], 1.0)
            eps_sb = const.tile([128, 1], f32, tag="eps")
            nc.vector.memset(eps_sb[:], EPS)
            warm_rhs = const.tile([128, 512], bf16, tag="warm_rhs")
            nc.vector.memset(warm_rhs[:], 0.0)
            tri_sb = const.tile([128, 128], bf16, tag="tri")

            # ---- resident weights / coefficients -------------------------
            wq_sb = persist.tile([128, kc_n, nhl * hd], bf16, tag="wq")
            wq_re = wq_d.rearrange("(kc p) m -> p kc m", p=128)
            wk_sb = persist.tile([128, kc_n, hd], bf16, tag="wk")
            wk_re = wk_d.rearrange("(kc p) m -> p kc m", p=128)
            wv_sb = persist.tile([128, kc_n, hd], bf16, tag="wv")
            wv_re = wv_d.rearrange("(kc p) m -> p kc m", p=128)
            wo_sb = persist.tile([128, nhl, d], bf16, tag="wo")
            wo_re = wo_d.rearrange("(h p) m -> p h m", p=128)

            m1q_sb = persist.tile([hd, s], bf16, tag="m1q")
            m2q_sb = persist.tile([hd, s], bf16, tag="m2q")
            m1k_sb = persist.tile([hd, s], bf16, tag="m1k")
            m2k_sb = persist.tile([hd, s], bf16, tag="m2k")

            # ---- persistent activations ---------------------------------
            qT_sb = [persist.tile([hd, s], bf16, tag=f"qT{h}", name=f"qT{h}")
                     for h in range(nhl)]
            kT_sb = persist.tile([hd, s], bf16, tag="kT")
            v_sb = persist.tile([128, kb_n, hd], bf16, tag="v")
            attT_sb = [persist.tile([hd, s], bf16, tag=f"attT{h}",
                                    name=f"attT{h}") for h in range(nhl)]

            xT_re = xT_d.rearrange("(kc p) n -> p kc n", p=128)

            # ---- norm + rope chain (deferred one tensor for overlap) -----
            def norm_rope_chain(q_ps, t, cs):
                # mean-of-squares broadcast to all partitions via the
                # all-ones stationary; rsqrt = Exp(-0.5 * Ln(mean + eps)),
                # keeping ACT in the natural_log_exp_and_others table set.
                sq = wa.tile([128, 512], bf16, tag="sq", name="sq")
                nc.scalar.activation(sq[:], q_ps[:], AF.Square)
                ssq = ps.tile([128, 512], f32, tag="mm", name="ssq", bufs=3)
                nc.tensor.matmul(ssq[:], ones128[:], sq[:])
                lnv = wa.tile([128, 512], f32, tag="lnv", name="lnv")
                nc.scalar.activation(
                    lnv[:], ssq[:], AF.Ln, scale=1.0 / hd, bias=eps_sb[:]
                )
                rb = wa.tile([128, 512], f32, tag="rb", name="rb")
                nc.scalar.activation(rb[:], lnv[:], AF.Exp, scale=-0.5)
                qn = wa.tile([128, 512], f32, tag="qn", name="qn")
                nc.vector.tensor_mul(qn[:], q_ps[:], rb[:])
                qs = wa.tile([128, 512], f32, tag="qs", name="qs")
                nc.sync.dma_start(qs[0:64, :], qn[64:128, :])
                nc.sync.dma_start(qs[64:128, :], qn[0:64, :])
                m1 = m1q_sb if t < nhl else m1k_sb
                m2 = m2q_sb if t < nhl else m2k_sb
                t1 = wa.tile([128, 512], f32, tag="t1", name="t1")
                nc.vector.tensor_mul(t1[:], qn[:], m1[:, cs])
                t2 = wa.tile([128, 512], f32, tag="t2", name="t2")
                nc.vector.tensor_mul(t2[:], qs[:], m2[:, cs])
                dest = qT_sb[t] if t < nhl else kT_sb
                nc.vector.tensor_add(dest[:, cs], t1[:], t2[:])

            # ---- causal flash attention for one 512-query block ----------
            def attn_round(qt, filler):
                nkb = 4 * qt + 4
                total_iters = nhl * nkb
                stride = max(1, total_iters // max(1, len(filler)))
                it = 0
                for h in range(nhl):
                    att_ps = ps.tile([128, 512], f32, tag="att", bufs=2,
                                     name=f"att{qt}_{h}")
                    l_ps = ps.tile([128, 512], f32, tag="lps", bufs=2,
                                   name=f"l{qt}_{h}")
                    s_tiles = {}

                    def emit_s(kb):
                        sp = ps.tile([128, 512], f32, tag="mm", name="s_ps",
                                     bufs=3)
                        r = kb - 4 * qt
                        c0 = 128 * r if r > 0 else 0
                        nc.tensor.matmul(
                            sp[:, c0:512],
                            kT_sb[:, kb * 128:(kb + 1) * 128],
                            qT_sb[h][:, qt * 512 + c0:(qt + 1) * 512],
                        )
                        s_tiles[kb] = sp

                    emit_s(0)
                    if nkb > 1:
                        emit_s(1)
                    for kb in range(nkb):
                        if kb + 2 < nkb:
                            emit_s(kb + 2)
                        sp = s_tiles.pop(kb)
                        r = kb - 4 * qt
                        c0 = 128 * r if r > 0 else 0
                        p = wb.tile([128, 512], bf16, tag="p", bufs=4)
                        nc.scalar.activation(
                            p[:, c0:512], sp[:, c0:512], AF.Exp, scale=SCALE
                        )
                        if r >= 0:
                            nc.vector.tensor_mul(
                                p[:, c0:c0 + 128], p[:, c0:c0 + 128], tri_sb[:]
                            )
                        nc.tensor.matmul(
                            att_ps[:, c0:512], v_sb[:, kb, :], p[:, c0:512],
                            start=(kb == 0), stop=(kb == nkb - 1),
                        )
                        nc.tensor.matmul(
                            l_ps[:, c0:512], ones128[:], p[:, c0:512],
                            start=(kb == 0), stop=(kb == nkb - 1),
                        )
                        it += 1
                        if filler and it % stride == 0:
                            filler.pop(0)()
                    rb_sb = wb.tile([128, 512], f32, tag="rba", bufs=2)
                    nc.vector.reciprocal_approx_fast(rb_sb[:], l_ps[:])
                    nc.vector.tensor_mul(
                        attT_sb[h][:, qt * 512:(qt + 1) * 512],
                        att_ps[:], rb_sb[:],
                    )

            # ---- output projection thunks for one 512-token block --------
            # Each thunk emits one y tile (4 accumulating MMs + DVE copy into
            # a [128, d] staging tile); the db==3 thunk also flushes the
            # staging tile to HBM in a single 1 MiB DMA. Thunks are popped
            # inside the next round's attention loop to fill the PE bubbles
            # left by ACT-paced exp.
            y_stage = {}

            def oproj_thunks(qt):
                thunks = []
                for tt in range(qt * 4, qt * 4 + 4):
                    def mk(tt):
                        def start_tt():
                            y_stage[tt] = wb.tile([128, d], f32, tag="ysb",
                                                  name=f"ysb{tt}", bufs=2)
                        return start_tt
                    thunks.append(mk(tt))
                    for db in range(d // 512):
                        def mk2(tt, db):
                            def emit_tile():
                                y_ps = ps.tile([128, 512], f32, tag="mm",
                                               name="y_ps", bufs=3)
                                for hh in range(nhl):
                                    nc.tensor.matmul(
                                        y_ps[:],
                                        attT_sb[hh][:, tt * 128:(tt + 1) * 128],
                                        wo_sb[:, hh, db * 512:(db + 1) * 512],
                                        start=(hh == 0), stop=(hh == nhl - 1),
                                    )
                                st = y_stage[tt]
                                nc.vector.tensor_copy(
                                    st[:, db * 512:(db + 1) * 512], y_ps[:]
                                )
                                if db == d // 512 - 1:
                                    nc.gpsimd.dma_start(
                                        y_d[tt * 128:(tt + 1) * 128, :], st[:]
                                    )
                            return emit_tile
                        thunks.append(mk2(tt, db))
                return thunks

            # ================= main pipeline ==============================
            pending = None
            xt_tiles = {}
            for nb in range(nb_n):
                cs = slice(nb * 512, (nb + 1) * 512)
                if nb == 0:
                    # Round-0 loads in criticality order: first x chunk and
                    # wq feed the very first matmuls; wo/tri are deferred.
                    xt = xtp.tile([128, kc_n, 512], bf16, tag="xt")
                    xt_tiles[0] = xt
                    nc.sync.dma_start(xt[:, 0:4, :], xT_re[:, 0:4, cs])
                    for g4 in range(4):
                        nc.gpsimd.dma_start(
                            wq_sb[:, 4 * g4:4 * g4 + 4, :],
                            wq_re[:, 4 * g4:4 * g4 + 4, :],
                        )
                    nc.sync.dma_start(xt[:, 4:8, :], xT_re[:, 4:8, cs])
                    nc.gpsimd.dma_start(wk_sb[:], wk_re[:])
                    nc.sync.dma_start(xt[:, 8:12, :], xT_re[:, 8:12, cs])
                    nc.gpsimd.dma_start(wv_sb[:], wv_re[:])
                    nc.sync.dma_start(xt[:, 12:16, :], xT_re[:, 12:16, cs])
                    nc.scalar.dma_start(m1k_sb[:], m1k_d[:])
                    nc.scalar.dma_start(m2k_sb[:], m2k_d[:])
                    nc.scalar.dma_start(m1q_sb[:], m1q_d[:])
                    nc.scalar.dma_start(m2q_sb[:], m2q_d[:])
                    nc.scalar.dma_start(tri_sb[:], tri_d[:])
                    # PE warmup while the first DMAs land
                    wps = ps.tile([128, 512], f32, tag="mm", bufs=3,
                                  name="wps")
                    for _ in range(14):
                        nc.tensor.matmul(wps[:], ones128[:], warm_rhs[:])
                xt = xt_tiles[nb]

                # k first (so attention can start as soon as possible),
                # then q heads; norm chain deferred one tensor
                for t in [nhl] + list(range(nhl)):
                    q_ps = ps.tile([128, 512], f32, tag="mm", bufs=3,
                                   name="q_ps")
                    for kc in range(kc_n):
                        if t < nhl:
                            lhsT = wq_sb[:, kc, t * hd:(t + 1) * hd]
                        else:
                            lhsT = wk_sb[:, kc, :]
                        nc.tensor.matmul(
                            q_ps[:], lhsT, xt[:, kc, :],
                            start=(kc == 0), stop=(kc == kc_n - 1),
                        )
                    if pending is not None:
                        norm_rope_chain(*pending)
                    pending = (q_ps, t, cs)

                # v: plain projection, token-major
                for tt in range(4):
                    v_ps = ps.tile([128, hd], f32, tag="vps", bufs=1)
                    for kc in range(kc_n):
                        nc.tensor.matmul(
                            v_ps[:],
                            xt[:, kc, tt * 128:(tt + 1) * 128],
                            wv_sb[:, kc, :],
                            start=(kc == 0), stop=(kc == kc_n - 1),
                        )
                    nc.vector.tensor_copy(v_sb[:, nb * 4 + tt, :], v_ps[:])

                # flush the last q chain so qT for this block is complete
                if pending is not None:
                    norm_rope_chain(*pending)
                    pending = None

                # prefetch next x block (gpsimd queue: the sync queue would
                # serialize the rope-swap DMAs behind the big transfers) and
                # wo before its first use in oproj(0)
                if nb + 1 < nb_n:
                    nxt = xtp.tile([128, kc_n, 512], bf16, tag="xt")
                    xt_tiles[nb + 1] = nxt
                    ncs = slice((nb + 1) * 512, (nb + 2) * 512)
                    for g in range(4):
                        nc.gpsimd.dma_start(
                            nxt[:, 4 * g:4 * g + 4, :],
                            xT_re[:, 4 * g:4 * g + 4, ncs],
                        )
                if nb == 0:
                    nc.gpsimd.dma_start(wo_sb[:], wo_re[:])

                filler = oproj_thunks(nb - 1) if nb > 0 else []
                attn_round(nb, filler)
                for th in filler:
                    th()

            for th in oproj_thunks(nb_n - 1):
                th()

    nc.compile()
    return nc


def _rope_coeffs(norm_w, s=S, hd=HD):
    """Coefficient tiles [hd, s] folding rope cos/sin + permuted norm weight."""
    perm = np.concatenate([np.arange(0, hd, 2), np.arange(1, hd, 2)])
    w = np.asarray(norm_w, np.float64)[perm]
    half = hd // 2
    pos = np.arange(s, dtype=np.float64)
    inv_freq = 1.0 / (THETA ** (np.arange(0, hd, 2, dtype=np.float64) / hd))
    ang = pos[None, :] * inv_freq[:, None]          # [half, s]
    cos, sin = np.cos(ang), np.sin(ang)
    m1 = np.empty((hd, s), np.float32)
    m2 = np.empty((hd, s), np.float32)
    m1[:half] = cos * w[:half, None]
    m1[half:] = cos * w[half:, None]
    m2[:half] = -sin * w[half:, None]
    m2[half:] = sin * w[:half, None]
    return m1, m2


def _host_prep(x, wq, wk, wv, wo, q_norm_w, k_norm_w):
    perm = np.concatenate([np.arange(0, HD, 2), np.arange(1, HD, 2)])
    m1q, m2q = _rope_coeffs(q_norm_w)
    m1k, m2k = _rope_coeffs(k_norm_w)
    tri = np.triu(np.ones((128, 128), np.float32)).astype(BF16)

    in_maps = []
    for c in range(8):
        b, g = c // 4, c % 4
        heads = range(NHL * g, NHL * g + NHL)
        wq_loc = np.concatenate(
            [wq[:, h * HD:(h + 1) * HD][:, perm] for h in heads], axis=1
        )
        in_maps.append({
            "xT": np.ascontiguousarray(x[b].T).astype(BF16),
            "wq": np.ascontiguousarray(wq_loc).astype(BF16),
            "wk": np.ascontiguousarray(wk[:, g * HD:(g + 1) * HD][:, perm]).astype(BF16),
            "wv": np.ascontiguousarray(wv[:, g * HD:(g + 1) * HD]).astype(BF16),
            "wo": np.ascontiguousarray(wo[NHL * g * HD:NHL * (g + 1) * HD, :]).astype(BF16),
            "m1q": m1q.astype(BF16), "m2q": m2q.astype(BF16),
            "m1k": m1k.astype(BF16), "m2k": m2k.astype(BF16),
            "tri": tri,
        })
    return in_maps


def _install_ntff_shim():
    import types
    if "antenv.axon_hooks" in sys.modules:
        return
    mod = types.ModuleType("antenv.axon_hooks")
    _hook = [None]
    mod.set_axon_ntff_profile_hook = lambda h: _hook.__setitem__(0, h)
    mod.get_axon_ntff_profile_hook = lambda: _hook[0]
    sys.modules["antenv.axon_hooks"] = mod
    try:
        from trn_agent_boot.trn_boot import _ntff_profile_via_ctypes
        mod.set_axon_ntff_profile_hook(
            _ntff_profile_via_ctypes("/opt/axon/libaxon_pjrt.so")
        )
    except Exception:
        pass


LAST_EXEC_NS = None


def kernel(x, wq, wk, wv, wo, q_norm_w, k_norm_w):
    global LAST_EXEC_NS
    from concourse import bass_utils

    x = np.asarray(x)
    if "nc" not in _CACHED:
        _CACHED["nc"] = build_nc()
    nc = _CACHED["nc"]

    in_maps = _host_prep(
        np.asarray(x, np.float32), np.asarray(wq, np.float32),
        np.asarray(wk, np.float32), np.asarray(wv, np.float32),
        np.asarray(wo, np.float32), np.asarray(q_norm_w, np.float32),
        np.asarray(k_norm_w, np.float32),
    )
    trace = bool(int(os.environ.get("BASS_KERNEL_TRACE", "0")))
    if trace:
        _install_ntff_shim()
    res = bass_utils.run_bass_kernel_spmd(
        nc, in_maps, core_ids=list(range(8)), trace=trace
    )
    LAST_EXEC_NS = res.exec_time_ns
    y = np.zeros((B, S, D), np.float32)
    for c in range(8):
        y[c // 4] += res.results[c]["y"]
    return y


# revision 28
# speedup vs baseline: 1.1350x; 1.1350x over previous
"""Trainium2 Bass kernel for GQA attention block (nn_Attention_81372450390110).

Module: y = AttnOut(x) with q/k RMSNorm + interleaved RoPE + causal GQA
(NH=16 q heads, KVH=4 kv heads, HD=128, D=2048, B=2, S=2048).

Sharding: 8 cores = 2 batches x 4 KV groups. Core c handles batch c//4 and
KV group c%4 (4 q heads + 1 kv head). Each core computes a full [S, D]
partial of the output projection (row-parallel over heads); the host sums
the 4 group-partials per batch.

Layout strategy (all feature-major, "transposed"):
  - host passes xT = x[b].T so the D contraction dim lands on partitions
  - qT/kT computed as [HD, S] directly (lhsT = weight chunk)
  - scores computed transposed: sT[k, q] = kT_blk.T @ qT_blk
  - softmax without max-subtraction (rmsnorm bounds |scores| <= sqrt(HD))
  - P@V computed transposed with V as the stationary operand:
      attT[hd, q] += v_blk.T.T @ pT_blk   (N=512 streams, no PE transposes)
    The softmax denominator l[q] accumulates in a parallel PSUM bank via an
    all-ones [128,128] stationary (every output partition = l), so the
    normalize is a single DVE mul with reciprocal_approx_fast - no
    broadcast matmul, no per-128-block transpose/copy chain.
  - rmsnorm row-sums use the same all-ones-stationary trick (ssq broadcast
    to all partitions in one matmul); rsqrt = Exp(-0.5*Ln(x)) so the whole
    kernel lives in one ACT table set (natural_log_exp_and_others), which
    lets projection rounds and attention rounds interleave without table
    reloads.
  - rounds pipeline: [oproj(nb-1)] [proj block nb + norm/rope] [attn qt=nb]
    so the PE never sees a phase barrier; x-block DMA for round nb+1
    overlaps attention nb.
  - RoPE+norm-weight folded into host-precomputed coefficient tiles, with
    an even/odd deinterleaving permutation baked into wq/wk columns
"""

import os
import sys

sys.path.insert(0, "/opt/trn_rl_repo")

import numpy as np
import ml_dtypes

BF16 = ml_dtypes.bfloat16

B = 2
S = 2048
D = 2048
NH = 16
KVH = 4
HD = 128
THETA = 10000.0
EPS = 1e-6
NHL = NH // KVH  # q heads per core (4)
SCALE = 1.0 / float(np.sqrt(HD))

_CACHED = {}


def build_nc(s=S, d=D, nhl=NHL, hd=HD):
    import concourse.mybir as mybir
    import concourse.tile as tile
    from concourse import bacc

    f32 = mybir.dt.float32
    bf16 = mybir.dt.bfloat16
    AF = mybir.ActivationFunctionType

    kc_n = d // 128          # contraction chunks for projections
    nb_n = s // 512          # 512-token blocks / rounds
    kb_n = s // 128          # k blocks (128 wide)

    nc = bacc.Bacc("TRN2", target_bir_lowering=False, debug=False)

    # x/wq/wk/wv arrive host-packed partition-major so every DMA reads
    # contiguous per-partition runs (see _host_prep)
    xT_d = nc.dram_tensor("xT", (128, nb_n * kc_n * 512), bf16,
                          kind="ExternalInput")
    wq_d = nc.dram_tensor("wq", (128, kc_n * nhl * hd), bf16,
                          kind="ExternalInput")
    wk_d = nc.dram_tensor("wk", (128, kc_n * hd), bf16, kind="ExternalInput")
    wv_d = nc.dram_tensor("wv", (128, kc_n * hd), bf16, kind="ExternalInput")
    wo_d = nc.dram_tensor("wo", (nhl * hd, d), bf16, kind="ExternalInput")
    m1q_d = nc.dram_tensor("m1q", (hd, s), bf16, kind="ExternalInput")
    m2q_d = nc.dram_tensor("m2q", (hd, s), bf16, kind="ExternalInput")
    m1k_d = nc.dram_tensor("m1k", (hd, s), bf16, kind="ExternalInput")
    m2k_d = nc.dram_tensor("m2k", (hd, s), bf16, kind="ExternalInput")
    tri_d = nc.dram_tensor("tri", (128, 128), bf16, kind="ExternalInput")
    y_d = nc.dram_tensor("y", (s, d), f32, kind="ExternalOutput")

    with tile.TileContext(nc) as tc, nc.allow_low_precision(
        reason="bf16 compute by design; fp32 accumulation in PSUM"
    ):
        with (
            tc.tile_pool(name="const", bufs=1) as const,
            tc.tile_pool(name="persist", bufs=1) as persist,
            tc.tile_pool(name="xtp", bufs=2) as xtp,
            tc.tile_pool(name="workA", bufs=3) as wa,
            tc.tile_pool(name="workB", bufs=3) as wb,
            tc.tile_pool(name="ps", bufs=1, space="PSUM") as ps,
        ):
            # Pin the ACT table set to natural_log_exp_and_others (id 6:
            # square+ln+exp) up front; the insert_act_table_loads fixpoint
            # then sees every activation covered and emits no further
            # (thrashing) reloads.
            nc.scalar.add_instruction(
                mybir.InstLoadActFuncSet(
                    name=nc.get_next_instruction_name(),
                    act_func_set_id=6,
                    ins=[], outs=[],
                )
            )

            # ---- constants -----------------------------------------------
            ones128 = const.tile([128, 128], bf16, tag="ones128")
            nc.vector.memset(ones128[:], 1.0)
            ones_k = const.tile([128, 1], bf16, tag="ones_k")
            nc.vector.memset(ones_k[:], 1.0)
            ones_1 = const.tile([1, 128], f32, tag="ones_1")
            nc.vector.memset(ones_1[:], 1.0)
            eps_sb = const.tile([128, 1], f32, tag="eps")
            nc.vector.memset(eps_sb[:], EPS)
            warm_rhs = const.tile([128, 512], bf16, tag="warm_rhs")
            nc.vector.memset(warm_rhs[:], 0.0)
            tri_sb = const.tile([128, 128], bf16, tag="tri")

            # ---- resident weights / coefficients -------------------------
            wq_sb = persist.tile([128, kc_n, nhl * hd], bf16, tag="wq")
            wk_sb = persist.tile([128, kc_n, hd], bf16, tag="wk")
            wv_sb = persist.tile([128, kc_n, hd], bf16, tag="wv")
            wo_sb = persist.tile([128, nhl, d], bf16, tag="wo")
            wo_re = wo_d.rearrange("(h p) m -> p h m", p=128)

            m1q_sb = persist.tile([hd, s], bf16, tag="m1q")
            m2q_sb = persist.tile([hd, s], bf16, tag="m2q")
            m1k_sb = persist.tile([hd, s], bf16, tag="m1k")
            m2k_sb = persist.tile([hd, s], bf16, tag="m2k")

            # ---- persistent activations ---------------------------------
            qT_sb = [persist.tile([hd, s], bf16, tag=f"qT{h}", name=f"qT{h}")
                     for h in range(nhl)]
            kT_sb = persist.tile([hd, s], bf16, tag="kT")
            v_sb = persist.tile([128, kb_n, hd], bf16, tag="v")
            attT_sb = [persist.tile([hd, s], bf16, tag=f"attT{h}",
                                    name=f"attT{h}") for h in range(nhl)]

            # ---- norm + rope chain (deferred one tensor for overlap) -----
            def norm_rope_chain(q_ps, t, cs):
                # mean-of-squares broadcast to all partitions via the
                # all-ones stationary; rsqrt = Exp(-0.5 * Ln(mean + eps)),
                # keeping ACT in the natural_log_exp_and_others table set.
                sq = wa.tile([128, 512], bf16, tag="sq", name="sq")
                nc.scalar.activation(sq[:], q_ps[:], AF.Square)
                ssq = ps.tile([128, 512], f32, tag="mm", name="ssq", bufs=4)
                nc.tensor.matmul(ssq[:], ones128[:], sq[:])
                lnv = wa.tile([128, 512], f32, tag="lnv", name="lnv")
                nc.scalar.activation(
                    lnv[:], ssq[:], AF.Ln, scale=1.0 / hd, bias=eps_sb[:]
                )
                rb = wa.tile([128, 512], f32, tag="rb", name="rb")
                nc.scalar.activation(rb[:], lnv[:], AF.Exp, scale=-0.5)
                # bf16 from here down: qT is stored bf16 anyway, and all-
                # bf16 operands let DVE run its 2x packed mode
                qn = wa.tile([128, 512], bf16, tag="qn", name="qn")
                nc.vector.tensor_mul(qn[:], q_ps[:], rb[:])
                qs = wa.tile([128, 512], bf16, tag="qs", name="qs")
                nc.sync.dma_start(qs[0:64, :], qn[64:128, :])
                nc.sync.dma_start(qs[64:128, :], qn[0:64, :])
                m1 = m1q_sb if t < nhl else m1k_sb
                m2 = m2q_sb if t < nhl else m2k_sb
                t1 = wa.tile([128, 512], bf16, tag="t1", name="t1")
                nc.vector.tensor_mul(t1[:], qn[:], m1[:, cs])
                t2 = wa.tile([128, 512], bf16, tag="t2", name="t2")
                nc.vector.tensor_mul(t2[:], qs[:], m2[:, cs])
                dest = qT_sb[t] if t < nhl else kT_sb
                nc.vector.tensor_add(dest[:, cs], t1[:], t2[:])

            # ---- causal flash attention for one 512-query block ----------
            def attn_round(qt, filler):
                nkb = 4 * qt + 4
                total_iters = nhl * nkb
                stride = max(1, total_iters // max(1, len(filler)))
                it = 0
                for h in range(nhl):
                    att_ps = ps.tile([128, 512], f32, tag="att", bufs=2,
                                     name=f"att{qt}_{h}")
                    # denominator accumulates broadcast to all partitions
                    # via the all-ones stationary, so the normalize is one
                    # reciprocal + one mul with no partition broadcast step
                    l_ps = ps.tile([128, 512], f32, tag="lps", bufs=1,
                                   name=f"l{qt}_{h}")
                    s_tiles = {}

                    def emit_s(kb):
                        sp = ps.tile([128, 512], f32, tag="mm", name="s_ps",
                                     bufs=4)
                        r = kb - 4 * qt
                        c0 = 128 * r if r > 0 else 0
                        nc.tensor.matmul(
                            sp[:, c0:512],
                            kT_sb[:, kb * 128:(kb + 1) * 128],
                            qT_sb[h][:, qt * 512 + c0:(qt + 1) * 512],
                        )
                        s_tiles[kb] = sp

                    emit_s(0)
                    if nkb > 1:
                        emit_s(1)
                    for kb in range(nkb):
                        if kb + 2 < nkb:
                            emit_s(kb + 2)
                        sp = s_tiles.pop(kb)
                        r = kb - 4 * qt
                        c0 = 128 * r if r > 0 else 0
                        p = wb.tile([128, 512], bf16, tag="p", bufs=4)
                        nc.scalar.activation(
                            p[:, c0:512], sp[:, c0:512], AF.Exp, scale=SCALE
                        )
                        if r >= 0:
                            nc.vector.tensor_mul(
                                p[:, c0:c0 + 128], p[:, c0:c0 + 128], tri_sb[:]
                            )
                        nc.tensor.matmul(
                            att_ps[:, c0:512], v_sb[:, kb, :], p[:, c0:512],
                            start=(kb == 0), stop=(kb == nkb - 1),
                        )
                        nc.tensor.matmul(
                            l_ps[:, c0:512], ones128[:], p[:, c0:512],
                            start=(kb == 0), stop=(kb == nkb - 1),
                        )
                        it += 1
                        if filler and it % stride == 0:
                            filler.pop(0)()
                    rb_sb = wb.tile([128, 512], f32, tag="rba", bufs=2)
                    nc.vector.reciprocal_approx_fast(rb_sb[:], l_ps[:])
                    nc.vector.tensor_mul(
                        attT_sb[h][:, qt * 512:(qt + 1) * 512],
                        att_ps[:], rb_sb[:],
                    )

            # ---- output projection thunks for one 512-token block --------
            # Each thunk emits one y tile (4 accumulating MMs + DVE copy into
            # a [128, d] staging tile); the db==3 thunk also flushes the
            # staging tile to HBM in a single 1 MiB DMA. Thunks are popped
            # inside the next round's attention loop to fill the PE bubbles
            # left by ACT-paced exp.
            y_stage = {}

            def oproj_thunks(qt):
                thunks = []
                for tt in range(qt * 4, qt * 4 + 4):
                    def mk(tt):
                        def start_tt():
                            y_stage[tt] = wb.tile([128, d], f32, tag="ysb",
                                                  name=f"ysb{tt}", bufs=2)
                        return start_tt
                    thunks.append(mk(tt))
                    for db in range(d // 512):
                        def mk2(tt, db):
                            def emit_tile():
                                y_ps = ps.tile([128, 512], f32, tag="mm",
                                               name="y_ps", bufs=4)
                                for hh in range(nhl):
                                    nc.tensor.matmul(
                                        y_ps[:],
                                        attT_sb[hh][:, tt * 128:(tt + 1) * 128],
                                        wo_sb[:, hh, db * 512:(db + 1) * 512],
                                        start=(hh == 0), stop=(hh == nhl - 1),
                                    )
                                st = y_stage[tt]
                                nc.vector.tensor_copy(
                                    st[:, db * 512:(db + 1) * 512], y_ps[:]
                                )
                                if db == d // 512 - 1:
                                    nc.gpsimd.dma_start(
                                        y_d[tt * 128:(tt + 1) * 128, :], st[:]
                                    )
                            return emit_tile
                        thunks.append(mk2(tt, db))
                return thunks

            # ================= main pipeline ==============================
            pending = None
            xt_tiles = {}
            for nb in range(nb_n):
                cs = slice(nb * 512, (nb + 1) * 512)
                if nb == 0:
                    # Round-0 loads in criticality order: wk + first x chunk
                    # feed the very first matmuls; wo/tri are deferred.
                    xt = xtp.tile([128, kc_n, 512], bf16, tag="xt")
                    xt_tiles[0] = xt
                    nc.gpsimd.dma_start(wk_sb[:, :, :], wk_d[:, :])
                    nc.sync.dma_start(xt[:, 0:4, :], xT_d[:, 0:2048])
                    for g4 in range(4):
                        nc.gpsimd.dma_start(
                            wq_sb[:, 4 * g4:4 * g4 + 4, :],
                            wq_d[:, g4 * 2048:(g4 + 1) * 2048],
                        )
                    nc.sync.dma_start(xt[:, 4:8, :], xT_d[:, 2048:4096])
                    nc.sync.dma_start(xt[:, 8:12, :], xT_d[:, 4096:6144])
                    nc.gpsimd.dma_start(wv_sb[:, :, :], wv_d[:, :])
                    nc.sync.dma_start(xt[:, 12:16, :], xT_d[:, 6144:8192])
                    nc.scalar.dma_start(m1k_sb[:], m1k_d[:])
                    nc.scalar.dma_start(m2k_sb[:], m2k_d[:])
                    nc.scalar.dma_start(m1q_sb[:], m1q_d[:])
                    nc.scalar.dma_start(m2q_sb[:], m2q_d[:])
                    nc.scalar.dma_start(tri_sb[:], tri_d[:])
                    # PE warmup while the first DMAs land
                    wps = ps.tile([128, 512], f32, tag="mm", bufs=4,
                                  name="wps")
                    for _ in range(14):
                        nc.tensor.matmul(wps[:], ones128[:], warm_rhs[:])
                xt = xt_tiles[nb]

                # k first (so attention can start as soon as possible),
                # then q heads; norm chain deferred one tensor
                for t in [nhl] + list(range(nhl)):
                    q_ps = ps.tile([128, 512], f32, tag="mm", bufs=4,
                                   name="q_ps")
                    for kc in range(kc_n):
                        if t < nhl:
                            lhsT = wq_sb[:, kc, t * hd:(t + 1) * hd]
                        else:
                            lhsT = wk_sb[:, kc, :]
                        nc.tensor.matmul(
                            q_ps[:], lhsT, xt[:, kc, :],
                            start=(kc == 0), stop=(kc == kc_n - 1),
                        )
                    if pending is not None:
                        norm_rope_chain(*pending)
                    pending = (q_ps, t, cs)

                # v: plain projection, token-major
                for tt in range(4):
                    v_ps = ps.tile([128, hd], f32, tag="vps", bufs=1)
                    for kc in range(kc_n):
                        nc.tensor.matmul(
                            v_ps[:],
                            xt[:, kc, tt * 128:(tt + 1) * 128],
                            wv_sb[:, kc, :],
                            start=(kc == 0), stop=(kc == kc_n - 1),
                        )
                    nc.vector.tensor_copy(v_sb[:, nb * 4 + tt, :], v_ps[:])

                # flush the last q chain so qT for this block is complete
                if pending is not None:
                    norm_rope_chain(*pending)
                    pending = None

                # prefetch next x block (gpsimd queue: the sync queue would
                # serialize the rope-swap DMAs behind the big transfers) and
                # wo before its first use in oproj(0)
                if nb + 1 < nb_n:
                    nxt = xtp.tile([128, kc_n, 512], bf16, tag="xt")
                    xt_tiles[nb + 1] = nxt
                    base = (nb + 1) * kc_n * 512
                    for g in range(4):
                        nc.gpsimd.dma_start(
                            nxt[:, 4 * g:4 * g + 4, :],
                            xT_d[:, base + g * 2048:base + (g + 1) * 2048],
                        )
                if nb == 0:
                    nc.gpsimd.dma_start(wo_sb[:], wo_re[:])

                filler = oproj_thunks(nb - 1) if nb > 0 else []
                attn_round(nb, filler)
                for th in filler:
                    th()

            for th in oproj_thunks(nb_n - 1):
                th()

    nc.compile()
    return nc


def _rope_coeffs(norm_w, s=S, hd=HD):
    """Coefficient tiles [hd, s] folding rope cos/sin + permuted norm weight."""
    perm = np.concatenate([np.arange(0, hd, 2), np.arange(1, hd, 2)])
    w = np.asarray(norm_w, np.float64)[perm]
    half = hd // 2
    pos = np.arange(s, dtype=np.float64)
    inv_freq = 1.0 / (THETA ** (np.arange(0, hd, 2, dtype=np.float64) / hd))
    ang = pos[None, :] * inv_freq[:, None]          # [half, s]
    cos, sin = np.cos(ang), np.sin(ang)
    m1 = np.empty((hd, s), np.float32)
    m2 = np.empty((hd, s), np.float32)
    m1[:half] = cos * w[:half, None]
    m1[half:] = cos * w[half:, None]
    m2[:half] = -sin * w[half:, None]
    m2[half:] = sin * w[:half, None]
    return m1, m2


def _pack_w(w):
    # [D, M] -> [128, kc*M]: partition-major so each partition's kc-chunks
    # are one contiguous run for the DMA
    dd, m = w.shape
    kc = dd // 128
    return np.ascontiguousarray(
        w.reshape(kc, 128, m).transpose(1, 0, 2).reshape(128, kc * m)
    )


def _host_prep(x, wq, wk, wv, wo, q_norm_w, k_norm_w):
    perm = np.concatenate([np.arange(0, HD, 2), np.arange(1, HD, 2)])
    m1q, m2q = _rope_coeffs(q_norm_w)
    m1k, m2k = _rope_coeffs(k_norm_w)
    tri = np.triu(np.ones((128, 128), np.float32)).astype(BF16)

    in_maps = []
    for c in range(8):
        b, g = c // 4, c % 4
        heads = range(NHL * g, NHL * g + NHL)
        wq_loc = np.concatenate(
            [wq[:, h * HD:(h + 1) * HD][:, perm] for h in heads], axis=1
        )
        # xT packed [128, nb*kc*512]: per round one contiguous [128, 8192]
        xT = x[b].T  # [D, S]
        xpk = (xT.reshape(D // 128, 128, S // 512, 512)
               .transpose(1, 2, 0, 3).reshape(128, -1))
        in_maps.append({
            "xT": np.ascontiguousarray(xpk).astype(BF16),
            "wq": _pack_w(wq_loc).astype(BF16),
            "wk": _pack_w(wk[:, g * HD:(g + 1) * HD][:, perm]).astype(BF16),
            "wv": _pack_w(wv[:, g * HD:(g + 1) * HD]).astype(BF16),
            "wo": np.ascontiguousarray(wo[NHL * g * HD:NHL * (g + 1) * HD, :]).astype(BF16),
            "m1q": m1q.astype(BF16), "m2q": m2q.astype(BF16),
            "m1k": m1k.astype(BF16), "m2k": m2k.astype(BF16),
            "tri": tri,
        })
    return in_maps


def _install_ntff_shim():
    import types
    if "antenv.axon_hooks" in sys.modules:
        return
    mod = types.ModuleType("antenv.axon_hooks")
    _hook = [None]
    mod.set_axon_ntff_profile_hook = lambda h: _hook.__setitem__(0, h)
    mod.get_axon_ntff_profile_hook = lambda: _hook[0]
    sys.modules["antenv.axon_hooks"] = mod
    try:
        from trn_agent_boot.trn_boot import _ntff_profile_via_ctypes
        mod.set_axon_ntff_profile_hook(
            _ntff_profile_via_ctypes("/opt/axon/libaxon_pjrt.so")
        )
    except Exception:
        pass


LAST_EXEC_NS = None


def kernel(x, wq, wk, wv, wo, q_norm_w, k_norm_w):
    global LAST_EXEC_NS
    from concourse import bass_utils

    x = np.asarray(x)
    if "nc" not in _CACHED:
        _CACHED["nc"] = build_nc()
    nc = _CACHED["nc"]

    in_maps = _host_prep(
        np.asarray(x, np.float32), np.asarray(wq, np.float32),
        np.asarray(wk, np.float32), np.asarray(wv, np.float32),
        np.asarray(wo, np.float32), np.asarray(q_norm_w, np.float32),
        np.asarray(k_norm_w, np.float32),
    )
    trace = bool(int(os.environ.get("BASS_KERNEL_TRACE", "0")))
    if trace:
        _install_ntff_shim()
    res = bass_utils.run_bass_kernel_spmd(
        nc, in_maps, core_ids=list(range(8)), trace=trace
    )
    LAST_EXEC_NS = res.exec_time_ns
    y = np.zeros((B, S, D), np.float32)
    for c in range(8):
        y[c // 4] += res.results[c]["y"]
    return y


# revision 40
# speedup vs baseline: 1.2024x; 1.0594x over previous
"""Trainium2 Bass kernel for GQA attention block (nn_Attention_81372450390110).

Module: y = AttnOut(x) with q/k RMSNorm + interleaved RoPE + causal GQA
(NH=16 q heads, KVH=4 kv heads, HD=128, D=2048, B=2, S=2048).

Sharding: 8 cores = 2 batches x 4 KV groups. Core c handles batch c//4 and
KV group c%4 (4 q heads + 1 kv head). Each core computes a full [S, D]
partial of the output projection (row-parallel over heads); the host sums
the 4 group-partials per batch.

Layout strategy (all feature-major, "transposed"):
  - host passes xT = x[b].T so the D contraction dim lands on partitions
  - qT/kT computed as [HD, S] directly (lhsT = weight chunk)
  - scores computed transposed: sT[k, q] = kT_blk.T @ qT_blk
  - softmax without max-subtraction (rmsnorm bounds |scores| <= sqrt(HD))
  - P@V computed transposed with V as the stationary operand:
      attT[hd, q] += v_blk.T.T @ pT_blk   (N=512 streams, no PE transposes)
    The softmax denominator l[q] accumulates in a parallel PSUM bank via an
    all-ones [128,128] stationary (every output partition = l), so the
    normalize is a single DVE mul with reciprocal_approx_fast - no
    broadcast matmul, no per-128-block transpose/copy chain.
  - rmsnorm row-sums use the same all-ones-stationary trick (ssq broadcast
    to all partitions in one matmul); rsqrt = Exp(-0.5*Ln(x)) so the whole
    kernel lives in one ACT table set (natural_log_exp_and_others), which
    lets projection rounds and attention rounds interleave without table
    reloads.
  - rounds pipeline: [oproj(nb-1)] [proj block nb + norm/rope] [attn qt=nb]
    so the PE never sees a phase barrier; x-block DMA for round nb+1
    overlaps attention nb.
  - RoPE+norm-weight folded into host-precomputed coefficient tiles, with
    an even/odd deinterleaving permutation baked into wq/wk columns
"""

import os
import sys

sys.path.insert(0, "/opt/trn_rl_repo")

import numpy as np
import ml_dtypes

BF16 = ml_dtypes.bfloat16

B = 2
S = 2048
D = 2048
NH = 16
KVH = 4
HD = 128
THETA = 10000.0
EPS = 1e-6
NHL = NH // KVH  # q heads per core (4)
SCALE = 1.0 / float(np.sqrt(HD))

_CACHED = {}


def build_nc(s=S, d=D, nhl=NHL, hd=HD):
    import concourse.mybir as mybir
    import concourse.tile as tile
    from concourse import bacc

    f32 = mybir.dt.float32
    bf16 = mybir.dt.bfloat16
    AF = mybir.ActivationFunctionType

    kc_n = d // 128          # contraction chunks for projections
    nb_n = s // 512          # 512-token blocks / rounds
    kb_n = s // 128          # k blocks (128 wide)

    nc = bacc.Bacc("TRN2", target_bir_lowering=False, debug=False)

    # x/wq/wk/wv arrive host-packed partition-major so every DMA reads
    # contiguous per-partition runs (see _host_prep)
    xT_d = nc.dram_tensor("xT", (128, nb_n * kc_n * 512), bf16,
                          kind="ExternalInput")
    wq_d = nc.dram_tensor("wq", (128, kc_n * nhl * hd), bf16,
                          kind="ExternalInput")
    wk_d = nc.dram_tensor("wk", (128, kc_n * hd), bf16, kind="ExternalInput")
    wv_d = nc.dram_tensor("wv", (128, kc_n * hd), bf16, kind="ExternalInput")
    wo_d = nc.dram_tensor("wo", (nhl * hd, d), bf16, kind="ExternalInput")
    m1q_d = nc.dram_tensor("m1q", (hd, s), bf16, kind="ExternalInput")
    m2q_d = nc.dram_tensor("m2q", (hd, s), bf16, kind="ExternalInput")
    m1k_d = nc.dram_tensor("m1k", (hd, s), bf16, kind="ExternalInput")
    m2k_d = nc.dram_tensor("m2k", (hd, s), bf16, kind="ExternalInput")
    tri_d = nc.dram_tensor("tri", (128, 128), bf16, kind="ExternalInput")
    y_d = nc.dram_tensor("y", (s, d), bf16, kind="ExternalOutput")

    with tile.TileContext(nc) as tc, nc.allow_low_precision(
        reason="bf16 compute by design; fp32 accumulation in PSUM"
    ):
        with (
            tc.tile_pool(name="const", bufs=1) as const,
            tc.tile_pool(name="persist", bufs=1) as persist,
            tc.tile_pool(name="xtp", bufs=2) as xtp,
            tc.tile_pool(name="workA", bufs=4) as wa,
            tc.tile_pool(name="workB", bufs=3) as wb,
            tc.tile_pool(name="ps", bufs=1, space="PSUM") as ps,
        ):
            # Pin the ACT table set to natural_log_exp_and_others (id 6:
            # square+ln+exp) up front; the insert_act_table_loads fixpoint
            # then sees every activation covered and emits no further
            # (thrashing) reloads.
            nc.scalar.add_instruction(
                mybir.InstLoadActFuncSet(
                    name=nc.get_next_instruction_name(),
                    act_func_set_id=6,
                    ins=[], outs=[],
                )
            )

            # ---- constants -----------------------------------------------
            ones128 = const.tile([128, 128], bf16, tag="ones128")
            nc.vector.memset(ones128[:], 1.0)
            ones_k = const.tile([128, 1], bf16, tag="ones_k")
            nc.vector.memset(ones_k[:], 1.0)
            ones_1 = const.tile([1, 128], f32, tag="ones_1")
            nc.vector.memset(ones_1[:], 1.0)
            eps_sb = const.tile([128, 1], f32, tag="eps")
            nc.vector.memset(eps_sb[:], EPS)
            warm_rhs = const.tile([128, 512], bf16, tag="warm_rhs")
            nc.vector.memset(warm_rhs[:], 0.0)
            tri_sb = const.tile([128, 128], bf16, tag="tri")

            # ---- resident weights / coefficients -------------------------
            wq_sb = persist.tile([128, nhl, kc_n, hd], bf16, tag="wq")
            wk_sb = persist.tile([128, kc_n, hd], bf16, tag="wk")
            wv_sb = persist.tile([128, kc_n, hd], bf16, tag="wv")
            wo_sb = persist.tile([128, nhl, d], bf16, tag="wo")
            wo_re = wo_d.rearrange("(h p) m -> p h m", p=128)

            m1q_sb = persist.tile([hd, s], bf16, tag="m1q")
            m2q_sb = persist.tile([hd, s], bf16, tag="m2q")
            m1k_sb = persist.tile([hd, s], bf16, tag="m1k")
            m2k_sb = persist.tile([hd, s], bf16, tag="m2k")

            # ---- persistent activations ---------------------------------
            qT_sb = [persist.tile([hd, s], bf16, tag=f"qT{h}", name=f"qT{h}")
                     for h in range(nhl)]
            kT_sb = persist.tile([hd, s], bf16, tag="kT")
            v_sb = persist.tile([128, kb_n, hd], bf16, tag="v")
            attT_sb = [persist.tile([hd, s], bf16, tag=f"attT{h}",
                                    name=f"attT{h}") for h in range(nhl)]

            # ---- norm + rope chain (deferred one tensor for overlap) -----
            def norm_rope_chain(q_ps, t, cs):
                # mean-of-squares broadcast to all partitions via the
                # all-ones stationary; rsqrt = Exp(-0.5 * Ln(mean + eps)),
                # keeping ACT in the natural_log_exp_and_others table set.
                sq = wa.tile([128, 512], bf16, tag="sq", name="sq")
                nc.scalar.activation(sq[:], q_ps[:], AF.Square)
                ssq = ps.tile([128, 512], f32, tag="mm", name="ssq", bufs=4)
                nc.tensor.matmul(ssq[:], ones128[:], sq[:])
                lnv = wa.tile([128, 512], f32, tag="lnv", name="lnv")
                nc.scalar.activation(
                    lnv[:], ssq[:], AF.Ln, scale=1.0 / hd, bias=eps_sb[:]
                )
                rb = wa.tile([128, 512], f32, tag="rb", name="rb")
                nc.scalar.activation(rb[:], lnv[:], AF.Exp, scale=-0.5)
                # bf16 from here down: qT is stored bf16 anyway, and all-
                # bf16 operands let DVE run its 2x packed mode
                qn = wa.tile([128, 512], bf16, tag="qn", name="qn")
                nc.vector.tensor_mul(qn[:], q_ps[:], rb[:])
                qs = wa.tile([128, 512], bf16, tag="qs", name="qs")
                nc.sync.dma_start(qs[0:64, :], qn[64:128, :])
                nc.sync.dma_start(qs[64:128, :], qn[0:64, :])
                m1 = m1q_sb if t < nhl else m1k_sb
                m2 = m2q_sb if t < nhl else m2k_sb
                t1 = wa.tile([128, 512], bf16, tag="t1", name="t1")
                nc.vector.tensor_mul(t1[:], qn[:], m1[:, cs])
                t2 = wa.tile([128, 512], bf16, tag="t2", name="t2")
                nc.vector.tensor_mul(t2[:], qs[:], m2[:, cs])
                dest = qT_sb[t] if t < nhl else kT_sb
                nc.vector.tensor_add(dest[:, cs], t1[:], t2[:])

            # ---- causal flash attention for one 512-query block ----------
            def attn_round(qt, filler):
                nkb = 4 * qt + 4
                total_iters = nhl * nkb
                stride = max(1, total_iters // max(1, len(filler)))
                it = 0
                for h in range(nhl):
                    att_ps = ps.tile([128, 512], f32, tag="att", bufs=2,
                                     name=f"att{qt}_{h}")
                    # denominator accumulates broadcast to all partitions
                    # via the all-ones stationary, so the normalize is one
                    # reciprocal + one mul with no partition broadcast step
                    l_ps = ps.tile([128, 512], f32, tag="lps", bufs=1,
                                   name=f"l{qt}_{h}")
                    s_tiles = {}

                    def emit_s(kb):
                        sp = ps.tile([128, 512], f32, tag="mm", name="s_ps",
                                     bufs=4)
                        r = kb - 4 * qt
                        c0 = 128 * r if r > 0 else 0
                        nc.tensor.matmul(
                            sp[:, c0:512],
                            kT_sb[:, kb * 128:(kb + 1) * 128],
                            qT_sb[h][:, qt * 512 + c0:(qt + 1) * 512],
                        )
                        s_tiles[kb] = sp

                    for i in range(min(3, nkb)):
                        emit_s(i)
                    full_tiles = []
                    pair_tiles = []
                    l_first = True
                    for kb in range(nkb):
                        if kb + 3 < nkb:
                            emit_s(kb + 3)
                        sp = s_tiles.pop(kb)
                        r = kb - 4 * qt
                        c0 = 128 * r if r > 0 else 0
                        p = wb.tile([128, 512], bf16, tag="p", bufs=6)
                        nc.scalar.activation(
                            p[:, c0:512], sp[:, c0:512], AF.Exp, scale=SCALE
                        )
                        if r >= 0:
                            nc.vector.tensor_mul(
                                p[:, c0:c0 + 128], p[:, c0:c0 + 128], tri_sb[:]
                            )
                        nc.tensor.matmul(
                            att_ps[:, c0:512], v_sb[:, kb, :], p[:, c0:512],
                            start=(kb == 0), stop=(kb == nkb - 1),
                        )
                        # denominator: reduce full-width p tiles 4-at-a-time
                        # with a cheap bf16 DVE add tree so each quad costs
                        # one ones-matmul instead of four (off-diag counts
                        # are multiples of 4, so quads always close)
                        if r < 0:
                            full_tiles.append(p)
                            if len(full_tiles) == 2:
                                pr = wb.tile([128, 512], bf16, tag="ppair",
                                             bufs=3)
                                nc.vector.tensor_add(
                                    pr[:], full_tiles[0][:], full_tiles[1][:]
                                )
                                full_tiles = []
                                pair_tiles.append(pr)
                                if len(pair_tiles) == 2:
                                    q4 = wb.tile([128, 512], bf16,
                                                 tag="pquad", bufs=2)
                                    nc.vector.tensor_add(
                                        q4[:], pair_tiles[0][:],
                                        pair_tiles[1][:],
                                    )
                                    pair_tiles = []
                                    nc.tensor.matmul(
                                        l_ps[:], ones128[:], q4[:],
                                        start=l_first, stop=False,
                                    )
                                    l_first = False
                        else:
                            for pt in pair_tiles:
                                nc.tensor.matmul(
                                    l_ps[:], ones128[:], pt[:],
                                    start=l_first, stop=False,
                                )
                                l_first = False
                            pair_tiles = []
                            nc.tensor.matmul(
                                l_ps[:, c0:512], ones128[:], p[:, c0:512],
                                start=l_first, stop=(kb == nkb - 1),
                            )
                            l_first = False
                        it += 1
                        if filler and it % stride == 0:
                            filler.pop(0)()
                    rb_sb = wb.tile([128, 512], f32, tag="rba", bufs=2)
                    nc.vector.reciprocal_approx_fast(rb_sb[:], l_ps[:])
                    nc.vector.tensor_mul(
                        attT_sb[h][:, qt * 512:(qt + 1) * 512],
                        att_ps[:], rb_sb[:],
                    )

            # ---- output projection thunks for one 512-token block --------
            # Each thunk emits one y tile (4 accumulating MMs + DVE copy into
            # a [128, d] staging tile); the db==3 thunk also flushes the
            # staging tile to HBM in a single 1 MiB DMA. Thunks are popped
            # inside the next round's attention loop to fill the PE bubbles
            # left by ACT-paced exp.
            y_stage = {}

            def oproj_thunks(qt):
                thunks = []
                for tt in range(qt * 4, qt * 4 + 4):
                    def mk(tt):
                        def start_tt():
                            y_stage[tt] = wb.tile([128, d], bf16, tag="ysb",
                                                  name=f"ysb{tt}", bufs=2)
                        return start_tt
                    thunks.append(mk(tt))
                    for db in range(d // 512):
                        def mk2(tt, db):
                            def emit_tile():
                                y_ps = ps.tile([128, 512], f32, tag="mm",
                                               name="y_ps", bufs=4)
                                for hh in range(nhl):
                                    nc.tensor.matmul(
                                        y_ps[:],
                                        attT_sb[hh][:, tt * 128:(tt + 1) * 128],
                                        wo_sb[:, hh, db * 512:(db + 1) * 512],
                                        start=(hh == 0), stop=(hh == nhl - 1),
                                    )
                                st = y_stage[tt]
                                nc.vector.tensor_copy(
                                    st[:, db * 512:(db + 1) * 512], y_ps[:]
                                )
                                if db == d // 512 - 1:
                                    nc.gpsimd.dma_start(
                                        y_d[tt * 128:(tt + 1) * 128, :], st[:]
                                    )
                            return emit_tile
                        thunks.append(mk2(tt, db))
                return thunks

            # ================= main pipeline ==============================
            pending = None
            xt_tiles = {}
            for nb in range(nb_n):
                cs = slice(nb * 512, (nb + 1) * 512)
                if nb == 0:
                    # Round-0 loads in criticality order: wk + first x chunk
                    # feed the very first matmuls; wo/tri are deferred.
                    xt = xtp.tile([128, kc_n, 512], bf16, tag="xt")
                    xt_tiles[0] = xt
                    # x chunks split across the sync and scalar queues so
                    # the two transfers run in parallel; weights on gpsimd;
                    # coefficients after the x halves they share a queue with
                    nc.gpsimd.dma_start(wk_sb[:, 0:8, :], wk_d[:, 0:1024])
                    nc.sync.dma_start(xt[:, 0:4, :], xT_d[:, 0:2048])
                    nc.scalar.dma_start(xt[:, 4:8, :], xT_d[:, 2048:4096])
                    nc.gpsimd.dma_start(wk_sb[:, 8:16, :], wk_d[:, 1024:2048])
                    # wq is packed head-major so each head's weights land
                    # just-in-time in consumption order (k, q0, q1, q2, q3)
                    nc.gpsimd.dma_start(wq_sb[:, 0, :, :], wq_d[:, 0:2048])
                    nc.gpsimd.dma_start(wq_sb[:, 1, :, :], wq_d[:, 2048:4096])
                    nc.sync.dma_start(xt[:, 8:12, :], xT_d[:, 4096:6144])
                    nc.scalar.dma_start(xt[:, 12:16, :], xT_d[:, 6144:8192])
                    nc.gpsimd.dma_start(wq_sb[:, 2, :, :], wq_d[:, 4096:6144])
                    nc.gpsimd.dma_start(wq_sb[:, 3, :, :], wq_d[:, 6144:8192])
                    nc.gpsimd.dma_start(wv_sb[:, :, :], wv_d[:, :])
                    nc.scalar.dma_start(m1k_sb[:], m1k_d[:])
                    nc.scalar.dma_start(m2k_sb[:], m2k_d[:])
                    nc.scalar.dma_start(m1q_sb[:], m1q_d[:])
                    nc.scalar.dma_start(m2q_sb[:], m2q_d[:])
                    nc.scalar.dma_start(tri_sb[:], tri_d[:])
                    # PE warmup while the first DMAs land
                    wps = ps.tile([128, 512], f32, tag="mm", bufs=4,
                                  name="wps")
                    for _ in range(18):
                        nc.tensor.matmul(wps[:], ones128[:], warm_rhs[:])
                xt = xt_tiles[nb]

                # k first (so attention can start as soon as possible),
                # then q heads; norm chain deferred one tensor
                for t in [nhl] + list(range(nhl)):
                    q_ps = ps.tile([128, 512], f32, tag="mm", bufs=4,
                                   name="q_ps")
                    for kc in range(kc_n):
                        if t < nhl:
                            lhsT = wq_sb[:, t, kc, :]
                        else:
                            lhsT = wk_sb[:, kc, :]
                        nc.tensor.matmul(
                            q_ps[:], lhsT, xt[:, kc, :],
                            start=(kc == 0), stop=(kc == kc_n - 1),
                        )
                    if pending is not None:
                        norm_rope_chain(*pending)
                    pending = (q_ps, t, cs)

                # v: plain projection, token-major
                for tt in range(4):
                    v_ps = ps.tile([128, hd], f32, tag="vps", bufs=1)
                    for kc in range(kc_n):
                        nc.tensor.matmul(
                            v_ps[:],
                            xt[:, kc, tt * 128:(tt + 1) * 128],
                            wv_sb[:, kc, :],
                            start=(kc == 0), stop=(kc == kc_n - 1),
                        )
                    nc.vector.tensor_copy(v_sb[:, nb * 4 + tt, :], v_ps[:])

                # flush the last q chain so qT for this block is complete
                if pending is not None:
                    norm_rope_chain(*pending)
                    pending = None

                # prefetch next x block (gpsimd queue: the sync queue would
                # serialize the rope-swap DMAs behind the big transfers) and
                # wo before its first use in oproj(0)
                if nb + 1 < nb_n:
                    nxt = xtp.tile([128, kc_n, 512], bf16, tag="xt")
                    xt_tiles[nb + 1] = nxt
                    base = (nb + 1) * kc_n * 512
                    for g in range(4):
                        nc.gpsimd.dma_start(
                            nxt[:, 4 * g:4 * g + 4, :],
                            xT_d[:, base + g * 2048:base + (g + 1) * 2048],
                        )
                if nb == 0:
                    nc.gpsimd.dma_start(wo_sb[:], wo_re[:])

                filler = oproj_thunks(nb - 1) if nb > 0 else []
                attn_round(nb, filler)
                for th in filler:
                    th()

            for th in oproj_thunks(nb_n - 1):
                th()

    nc.compile()
    return nc


def _rope_coeffs(norm_w, s=S, hd=HD):
    """Coefficient tiles [hd, s] folding rope cos/sin + permuted norm weight."""
    perm = np.concatenate([np.arange(0, hd, 2), np.arange(1, hd, 2)])
    w = np.asarray(norm_w, np.float64)[perm]
    half = hd // 2
    pos = np.arange(s, dtype=np.float64)
    inv_freq = 1.0 / (THETA ** (np.arange(0, hd, 2, dtype=np.float64) / hd))
    ang = pos[None, :] * inv_freq[:, None]          # [half, s]
    cos, sin = np.cos(ang), np.sin(ang)
    m1 = np.empty((hd, s), np.float32)
    m2 = np.empty((hd, s), np.float32)
    m1[:half] = cos * w[:half, None]
    m1[half:] = cos * w[half:, None]
    m2[:half] = -sin * w[half:, None]
    m2[half:] = sin * w[:half, None]
    return m1, m2


def _pack_w(w):
    # [D, M] -> [128, kc*M]: partition-major so each partition's kc-chunks
    # are one contiguous run for the DMA
    dd, m = w.shape
    kc = dd // 128
    return np.ascontiguousarray(
        w.reshape(kc, 128, m).transpose(1, 0, 2).reshape(128, kc * m)
    )


def _pack_w_hmaj(w):
    # [D, NHL*HD] -> [128, h*kc*hd]: head-major so each head's weights are
    # one contiguous per-partition run (loaded per-head, just-in-time)
    dd, m = w.shape
    kc = dd // 128
    return np.ascontiguousarray(
        w.reshape(kc, 128, NHL, HD).transpose(1, 2, 0, 3).reshape(128, -1)
    )


def _host_prep(x, wq, wk, wv, wo, q_norm_w, k_norm_w):
    perm = np.concatenate([np.arange(0, HD, 2), np.arange(1, HD, 2)])
    m1q, m2q = _rope_coeffs(q_norm_w)
    m1k, m2k = _rope_coeffs(k_norm_w)
    tri = np.triu(np.ones((128, 128), np.float32)).astype(BF16)

    in_maps = []
    for c in range(8):
        b, g = c // 4, c % 4
        heads = range(NHL * g, NHL * g + NHL)
        wq_loc = np.concatenate(
            [wq[:, h * HD:(h + 1) * HD][:, perm] for h in heads], axis=1
        )
        # xT packed [128, nb*kc*512]: per round one contiguous [128, 8192]
        xT = x[b].T  # [D, S]
        xpk = (xT.reshape(D // 128, 128, S // 512, 512)
               .transpose(1, 2, 0, 3).reshape(128, -1))
        in_maps.append({
            "xT": np.ascontiguousarray(xpk).astype(BF16),
            "wq": _pack_w_hmaj(wq_loc).astype(BF16),
            "wk": _pack_w(wk[:, g * HD:(g + 1) * HD][:, perm]).astype(BF16),
            "wv": _pack_w(wv[:, g * HD:(g + 1) * HD]).astype(BF16),
            "wo": np.ascontiguousarray(wo[NHL * g * HD:NHL * (g + 1) * HD, :]).astype(BF16),
            "m1q": m1q.astype(BF16), "m2q": m2q.astype(BF16),
            "m1k": m1k.astype(BF16), "m2k": m2k.astype(BF16),
            "tri": tri,
        })
    return in_maps


def _install_ntff_shim():
    import types
    if "antenv.axon_hooks" in sys.modules:
        return
    mod = types.ModuleType("antenv.axon_hooks")
    _hook = [None]
    mod.set_axon_ntff_profile_hook = lambda h: _hook.__setitem__(0, h)
    mod.get_axon_ntff_profile_hook = lambda: _hook[0]
    sys.modules["antenv.axon_hooks"] = mod
    try:
        from trn_agent_boot.trn_boot import _ntff_profile_via_ctypes
        mod.set_axon_ntff_profile_hook(
            _ntff_profile_via_ctypes("/opt/axon/libaxon_pjrt.so")
        )
    except Exception:
        pass


LAST_EXEC_NS = None


def kernel(x, wq, wk, wv, wo, q_norm_w, k_norm_w):
    global LAST_EXEC_NS
    from concourse import bass_utils

    x = np.asarray(x)
    if "nc" not in _CACHED:
        _CACHED["nc"] = build_nc()
    nc = _CACHED["nc"]

    in_maps = _host_prep(
        np.asarray(x, np.float32), np.asarray(wq, np.float32),
        np.asarray(wk, np.float32), np.asarray(wv, np.float32),
        np.asarray(wo, np.float32), np.asarray(q_norm_w, np.float32),
        np.asarray(k_norm_w, np.float32),
    )
    trace = bool(int(os.environ.get("BASS_KERNEL_TRACE", "0")))
    if trace:
        _install_ntff_shim()
    res = bass_utils.run_bass_kernel_spmd(
        nc, in_maps, core_ids=list(range(8)), trace=trace
    )
    LAST_EXEC_NS = res.exec_time_ns
    y = np.zeros((B, S, D), np.float32)
    for c in range(8):
        y[c // 4] += np.asarray(res.results[c]["y"], np.float32)
    return y
